# revision 1
# baseline (speedup 1.0000x reference)
"""Trainium2 Bass kernel for nn_CA1AttentionGate.

Computes, for full inputs (B=1, S=8192, H=1024, F=128, K=2):
    temporal = relu(t @ Wt1 + bt1) @ Wt2 + bt2          [K,F]
    mem      = dg_features + temporal                    [K,F]
    qmean    = query.mean(axis=1)                        [1,H]
    score_k  = tanh([mem_k ; qmean] @ Wa1 + ba1) @ Wa2 + ba2
    w_k      = sigmoid(score_k)
    g_k      = mem_k @ Wg + bg                           [K,H]
    row[s]   = (1/K) * sum_k w_k * (g_k . key[s])        [S]
    out      = broadcast(row) -> [1,1,S,S]

Sharding: sequence-parallel over the key/seq axis across 8 cores.  Each
core computes the final gate row for its 1024 key positions and writes
its dense [8192, 1024] column slab of the output.  The only cross-core
quantity is qmean: each core reduces its query shard and a 4KB AllReduce
completes the mean (fallback variant replicates the full query read).
"""

import os

import numpy as np

SEQ = 8192
H = 1024
F = 128
K = 2
NCORES = 8
SHARD = SEQ // NCORES  # 1024
NT = SHARD // 128  # 8 key tiles per shard

_PROG_CACHE = {}


def _build(use_collective: bool):
    import concourse.bacc as bacc
    import concourse.bass as bass
    import concourse.tile as tile
    from concourse import mybir
    from concourse.tile_rust import add_dep_helper

    AF = mybir.ActivationFunctionType
    ALU = mybir.AluOpType
    f32 = mybir.dt.float32

    nc = bacc.Bacc(
        "TRN2",
        target_bir_lowering=False,
        debug=False,
        num_devices=NCORES,
    )

    def din(name, shape):
        return nc.dram_tensor(name, list(shape), f32, kind="ExternalInput").ap()

    q_rows = SHARD if use_collective else SEQ
    qs = din("qs", (q_rows, H))
    ks = din("ks", (SHARD, H))
    dg = din("dg", (K, F))
    ts = din("ts", (K,))
    Wt1 = din("Wt1", (1, F // 4))
    bt1 = din("bt1", (F // 4,))
    Wt2 = din("Wt2", (F // 4, F))
    bt2 = din("bt2", (F,))
    Wa1 = din("Wa1", (F + H, F))
    ba1 = din("ba1", (F,))
    Wa2 = din("Wa2", (F, 1))
    ba2 = din("ba2", (1,))
    Wg = din("Wg", (F, H))
    bg = din("bg", (H,))
    # column of 1/SEQ: the qsum partition-reduce matmul yields the scaled
    # mean contribution directly
    scale_col = din("scale_col", (128, 1))
    out = nc.dram_tensor("out", [SEQ, SHARD], f32, kind="ExternalOutput").ap()

    def bcast(ap, n):
        # replicate a DRAM row across n partitions (stride-0 partition dim)
        return bass.AP(tensor=ap.tensor, offset=ap.offset, ap=[[0, n]] + list(ap.ap))

    def col(ap, n):
        # load a flat [n] DRAM vector as an [n, 1] column
        return bass.AP(tensor=ap.tensor, offset=ap.offset, ap=[[1, n], [n, 1]])

    with tile.TileContext(nc) as tc:
        with (
            tc.tile_pool(name="consts", bufs=1) as cp,
            tc.tile_pool(name="work", bufs=1) as wp,
            tc.tile_pool(name="qstream", bufs=8) as qp,
            tc.tile_pool(name="scratch", bufs=3) as sp,
            tc.tile_pool(name="psum_small", bufs=2, space="PSUM") as pps,
            tc.tile_pool(name="psum_big", bufs=3, space="PSUM") as ppb,
            tc.tile_pool(name="dram", bufs=1, space="DRAM") as dp,
        ):
            # ---- constant / weight loads (sync DGE ring) ---------------
            sc_c = cp.tile([128, 1], f32)
            nc.sync.dma_start(sc_c, scale_col)
            # ---- query shard DMAs get the wire first -------------------
            # (every weight/const below has >=10us of slack; the query
            # stream feeds the collective and must not queue behind them)
            nq = q_rows // 128
            qv = qs.rearrange("(t p) h -> t p h", p=128)
            qtiles = []
            q_insts = []
            for i in range(nq):
                qt = qp.tile([128, H], f32, tag="qt")
                q_insts.append(nc.sync.dma_start(qt, qv[i]))
                qtiles.append(qt)

            Wt2_sb = cp.tile([F // 4, F], f32)
            nc.sync.dma_start(Wt2_sb, Wt2)
            Wa1m_sb = cp.tile([128, 128], f32)
            nc.sync.dma_start(Wa1m_sb, Wa1[0:F, :])
            # qmean rows of Wa1 re-paired to the interleaved qmT layout:
            # chunk c pairs with rows {128 + i*8 + c}
            Wa1q_sb = cp.tile([128, 8, 128], f32)
            nc.sync.dma_start(
                Wa1q_sb, Wa1[F : F + H, :].rearrange("(i c) f -> i c f", c=8)
            )
            Wa2_sb = cp.tile([F, 1], f32)
            nc.sync.dma_start(Wa2_sb, Wa2)
            Wg_sb = cp.tile([F, H], f32)
            nc.sync.dma_start(Wg_sb, Wg)
            dgT_sb = cp.tile([F, K], f32)
            nc.sync.dma_start(dgT_sb, dg.rearrange("k f -> f k"))
            tb_sb = cp.tile([F // 4, K], f32)
            nc.sync.dma_start(tb_sb, bcast(ts, F // 4))
            Wt1T_sb = cp.tile([F // 4, 1], f32)
            nc.sync.dma_start(Wt1T_sb, col(Wt1, F // 4))
            bt1T_sb = cp.tile([F // 4, 1], f32)
            nc.sync.dma_start(bt1T_sb, col(bt1, F // 4))
            bt2T_sb = cp.tile([F, 1], f32)
            nc.sync.dma_start(bt2T_sb, col(bt2, F))
            ba1T_sb = cp.tile([F, 1], f32)
            nc.sync.dma_start(ba1T_sb, col(ba1, F))
            ba2b_sb = cp.tile([1, 1], f32)
            nc.sync.dma_start(ba2b_sb, bcast(ba2, 1))
            bg_sb = cp.tile([1, H], f32)
            nc.sync.dma_start(bg_sb, bg.rearrange("(a h) -> a h", a=1))

            # warm the ACT function tables used late in the critical path
            warm1 = cp.tile([1, 1], f32)
            nc.scalar.activation(warm1, sc_c[0:1, :], AF.Tanh)
            warm2 = cp.tile([1, 1], f32)
            nc.scalar.activation(warm2, sc_c[0:1, :], AF.Sigmoid)
            # key shard: interleaved, ktiles[j][p, :] = ks[p*NT + j, :];
            # explicitly ordered after the query stream so the query mean
            # (-> collective) is not starved of read bandwidth
            kv = ks.rearrange("(p t) h -> p t h", t=NT)
            ktiles = []
            for j in range(NT):
                kt = cp.tile([128, H], f32, tag=f"ks{j}")
                ki = nc.sync.dma_start(kt, kv[:, j, :])
                add_dep_helper(ki.ins, q_insts[-1].ins,
                               reason="key reads after query")
                ktiles.append(kt)

            # ---- query accumulate on DVE: head of the critical path ----
            qacc = wp.tile([128, H], f32)
            for i in range(nq):
                if i == 0:
                    nc.vector.tensor_copy(qacc, qtiles[i])
                else:
                    nc.vector.tensor_add(qacc, qacc, qtiles[i])

            # ---- qmean partial (PE first) -> collective ----------------
            qsum_ps = ppb.tile([1, H], f32, tag="big")
            nc.tensor.matmul(
                qsum_ps[:, 0:512], lhsT=sc_c, rhs=qacc[:, 0:512],
                start=True, stop=True,
            )
            nc.tensor.matmul(
                qsum_ps[:, 512:1024], lhsT=sc_c, rhs=qacc[:, 512:1024],
                start=True, stop=True,
            )
            qpart_sb = wp.tile([1, H], f32)
            nc.scalar.copy(qpart_sb, qsum_ps)
            if use_collective:
                cc_in = dp.tile([1, H], f32)
                cc_out = dp.tile([NCORES, H], f32)
                nc.scalar.dma_start(cc_in, qpart_sb)
                nc.gpsimd.collective_compute(
                    "AllGather",
                    ALU.bypass,
                    replica_groups=[list(range(NCORES))],
                    ins=[cc_in.opt()],
                    outs=[cc_out.opt()],
                )
                # park the gather-result load on the (idle) sync ring
                qmTd8 = wp.tile([128, NCORES, 8], f32)
                nc.sync.dma_start(
                    qmTd8, cc_out[:, :].rearrange("d (p c) -> p d c", c=8)
                )

            # ---- temporal MLP -> memT [F, K] ---------------------------
            h1T = wp.tile([F // 4, K], f32)
            nc.vector.tensor_scalar_mul(h1T, tb_sb, Wt1T_sb)
            nc.vector.tensor_scalar_add(h1T, h1T, bt1T_sb)
            nc.vector.tensor_relu(h1T, h1T)
            tT_ps = pps.tile([F, K], f32, tag="small")
            nc.tensor.matmul(tT_ps, lhsT=Wt2_sb, rhs=h1T, start=True, stop=True)
            memT_sb = wp.tile([F, K], f32)
            nc.scalar.activation(memT_sb, tT_ps, AF.Identity, bias=bt2T_sb, scale=1.0)
            nc.vector.tensor_add(memT_sb, memT_sb, dgT_sb)

            # ---- gate rows g_k = mem_k @ Wg + bg  [1, H] ---------------
            def g_row(k):
                g_ps = ppb.tile([1, H], f32, tag="big")
                nc.tensor.matmul(g_ps[:, 0:512], lhsT=memT_sb[:, k : k + 1],
                                 rhs=Wg_sb[:, 0:512], start=True, stop=True)
                nc.tensor.matmul(g_ps[:, 512:1024], lhsT=memT_sb[:, k : k + 1],
                                 rhs=Wg_sb[:, 512:1024], start=True, stop=True)
                return g_ps

            g0_ps = g_row(0)
            g0_sb = wp.tile([1, H], f32, tag="g0r")
            nc.vector.tensor_add(g0_sb, g0_ps, bg_sb)
            gb0 = wp.tile([128, H], f32, tag="gb0")
            nc.gpsimd.partition_broadcast(gb0[:, :], g0_sb[:, :])
            g1_ps = g_row(1)

            # ---- matvec: DVE muls, ACT accumulate-reductions -----------
            # rcc[p, j, k] = sum_h g_k[h] * ks[p*NT+j, h]
            rcc = wp.tile([128, NT, K], f32)

            def matvec(k, gb, js):
                for j in js:
                    prod = sp.tile([128, H], f32, tag="prod")
                    nc.vector.tensor_mul(prod, ktiles[j], gb)
                    junk = sp.tile([128, H], f32, tag="junk")
                    nc.scalar.activation(
                        junk, prod, AF.Copy,
                        accum_out=rcc[:, j, k : k + 1],
                    )

            matvec(0, gb0, range(4))

            # finish g1 mid-stream (its inputs are ready by now)
            g1_sb = wp.tile([1, H], f32, tag="g1r")
            nc.vector.tensor_add(g1_sb, g1_ps, bg_sb)
            gb1 = wp.tile([128, H], f32, tag="gb1")
            nc.gpsimd.partition_broadcast(gb1[:, :], g1_sb[:, :])

            matvec(0, gb0, range(4, NT))
            matvec(1, gb1, range(NT))

            # reshape both anchors at once to an interleaved row:
            # rTi[0, 2*s + k] = r_k[s]   (s = p*NT + j)
            rTi = wp.tile([1, K * SHARD], f32)
            nc.sync.dma_start(rTi[:, :], rcc[:, :, :])

            # ---- post-collective: qmT, scorer, weights -----------------
            # qmT[p, c] = qmean[p*8 + c]  (interleaved reshape layout)
            qmT = wp.tile([128, 8], f32)
            if use_collective:
                # sum gathered partials over d ([p, c, d] view, reduce X)
                nc.vector.tensor_reduce(
                    qmT, qmTd8[:, :, :].rearrange("p d c -> p c d"),
                    axis=mybir.AxisListType.X, op=ALU.add,
                )
            else:
                nc.scalar.dma_start(qmT, qpart_sb[:, :])
            qmTd = wp.tile([128, 8, K], f32)
            nc.vector.tensor_copy(qmTd[:, :, 0:1], qmT[:, :].rearrange("p c -> p c ()"))
            nc.vector.tensor_copy(qmTd[:, :, 1:2], qmT[:, :].rearrange("p c -> p c ()"))
            haT_ps = pps.tile([F, K], f32, tag="small")
            nc.tensor.matmul(haT_ps, lhsT=Wa1m_sb, rhs=memT_sb,
                             start=True, stop=False)
            for c in range(8):
                nc.tensor.matmul(haT_ps, lhsT=Wa1q_sb[:, c, :],
                                 rhs=qmTd[:, c, :], start=False, stop=(c == 7))
            aT_sb = wp.tile([F, K], f32)
            nc.scalar.activation(aT_sb, haT_ps, AF.Tanh, bias=ba1T_sb, scale=1.0)
            scoreT_ps = pps.tile([1, K], f32, tag="small")
            nc.tensor.matmul(scoreT_ps, lhsT=Wa2_sb, rhs=aT_sb, start=True, stop=True)
            wvT_sb = wp.tile([1, K], f32)
            nc.scalar.activation(wvT_sb, scoreT_ps, AF.Sigmoid, bias=ba2b_sb, scale=1.0)
            nc.scalar.mul(wvT_sb, wvT_sb, 1.0 / K)

            # ---- combine anchors in row space, then one broadcast ------
            rt = rTi[:, :]
            r_ev = bass.AP(tensor=rt.tensor, offset=rt.offset,
                           ap=[[K * SHARD, 1], [K, SHARD]])
            r_od = bass.AP(tensor=rt.tensor, offset=rt.offset + 1,
                           ap=[[K * SHARD, 1], [K, SHARD]])
            o_row = wp.tile([1, SHARD], f32)
            o_tmp = wp.tile([1, SHARD], f32)
            nc.vector.tensor_scalar_mul(o_row, r_ev, wvT_sb[0:1, 0:1])
            nc.vector.tensor_scalar_mul(o_tmp, r_od, wvT_sb[0:1, 1:2])
            nc.vector.tensor_add(o_row, o_row, o_tmp)
            out_sb = wp.tile([128, SHARD], f32)
            nc.gpsimd.partition_broadcast(out_sb[:, :], o_row[:, :])

            # ---- output: 64 x [128 rows, SHARD cols], all rows = row ---
            outv = out.rearrange("(b p) n -> b p n", p=128)
            for b in range(SEQ // 128):
                nc.sync.dma_start(outv[b], out_sb)

    nc.compile()
    return nc


def _get_prog(use_collective: bool):
    key = bool(use_collective)
    if key not in _PROG_CACHE:
        _PROG_CACHE[key] = _build(key)
    return _PROG_CACHE[key]


def _make_in_maps(inputs, use_collective: bool):
    q = np.ascontiguousarray(np.asarray(inputs["query"], np.float32)[0])  # [S,H]
    k = np.ascontiguousarray(np.asarray(inputs["key"], np.float32)[0])  # [S,H]
    common = {
        "dg": np.ascontiguousarray(np.asarray(inputs["dg_features"], np.float32)),
        "ts": np.ascontiguousarray(np.asarray(inputs["timestamps"], np.float32)),
        "Wt1": np.ascontiguousarray(np.asarray(inputs["Wt1"], np.float32)),
        "bt1": np.ascontiguousarray(np.asarray(inputs["bt1"], np.float32)),
        "Wt2": np.ascontiguousarray(np.asarray(inputs["Wt2"], np.float32)),
        "bt2": np.ascontiguousarray(np.asarray(inputs["bt2"], np.float32)),
        "Wa1": np.ascontiguousarray(np.asarray(inputs["Wa1"], np.float32)),
        "ba1": np.ascontiguousarray(np.asarray(inputs["ba1"], np.float32)),
        "Wa2": np.ascontiguousarray(np.asarray(inputs["Wa2"], np.float32)),
        "ba2": np.ascontiguousarray(np.asarray(inputs["ba2"], np.float32)),
        "Wg": np.ascontiguousarray(np.asarray(inputs["Wg"], np.float32)),
        "bg": np.ascontiguousarray(np.asarray(inputs["bg"], np.float32)),
        "scale_col": np.full((128, 1), 1.0 / 8192.0, np.float32),
    }
    in_maps = []
    for d in range(NCORES):
        m = dict(common)
        m["ks"] = np.ascontiguousarray(k[d * SHARD : (d + 1) * SHARD])
        if use_collective:
            m["qs"] = np.ascontiguousarray(q[d * SHARD : (d + 1) * SHARD])
        else:
            m["qs"] = q
        in_maps.append(m)
    return in_maps


def _run(inputs, use_collective: bool, trace: bool = False):
    from concourse.bass_utils import run_bass_kernel_spmd

    nc = _get_prog(use_collective)
    in_maps = _make_in_maps(inputs, use_collective)
    res = run_bass_kernel_spmd(
        nc, in_maps, core_ids=list(range(NCORES)), trace=trace
    )
    full = np.empty((1, 1, SEQ, SEQ), np.float32)
    for d in range(NCORES):
        full[0, 0, :, d * SHARD : (d + 1) * SHARD] = res.results[d]["out"]
    return full, res


def kernel(**inputs) -> np.ndarray:
    use_collective = os.environ.get("CA1_NO_COLLECTIVE", "0") != "1"
    try:
        full, _ = _run(inputs, use_collective)
        return full
    except Exception:
        if not use_collective:
            raise
        # fall back to the zero-communication variant (replicated query)
        _PROG_CACHE.pop(True, None)
        full, _ = _run(inputs, False)
        return full



# revision 2
# speedup vs baseline: 3.4974x; 3.4974x over previous
"""Trainium2 Bass kernel for nn_CA1AttentionGate.

Computes, for full inputs (B=1, S=8192, H=1024, F=128, K=2):
    temporal = relu(t @ Wt1 + bt1) @ Wt2 + bt2          [K,F]
    mem      = dg_features + temporal                    [K,F]
    qmean    = query.mean(axis=1)                        [1,H]
    score_k  = tanh([mem_k ; qmean] @ Wa1 + ba1) @ Wa2 + ba2
    w_k      = sigmoid(score_k)
    g_k      = mem_k @ Wg + bg                           [K,H]
    row[s]   = (1/K) * sum_k w_k * (g_k . key[s])        [S]
    out      = broadcast(row) -> [1,1,S,S]

Sharding: sequence-parallel over the key/seq axis across 8 cores.  Every
row of the [S,S] output is the same vector, so each core computes only
its 1024-entry slice of that broadcast row from its key shard (the
sharding_hint's "slice of the broadcast row") and the host unshard step
expands the gathered row to the full output.  The only cross-core
quantity is qmean: each core reduces its query shard to per-chunk column
sums and a 4KB AllGather completes the mean (fallback variant replicates
the full query read).

Inputs are staged transposed ([H, shard]) and in bf16 so the query
column-sums are free-axis DVE reduces and the g.key matvec is a chain of
bf16 PE matmuls accumulating in PSUM; 1/K is folded into Wg/bg and 1/S
into the qmean rows of Wa1 on the host.
"""

import os

import numpy as np

SEQ = 8192
H = 1024
F = 128
K = 2
NCORES = 8
SHARD = SEQ // NCORES  # 1024
NCH = H // 128  # 8 h-chunks of 128

# packed f32 constant tensor [128, NP_] column layout
_C_WT1 = 0
_C_BT1 = 1
_C_TS = 2
_C_WT2 = 4
_C_BT2 = 132
_C_DGT = 133
_C_BA1 = 135
_C_BA2 = 136
_C_BGT = 137
_C_WA2 = 145
_C_WA1M = 146
_C_WA1Q = 274
_C_WG = 274 + H
NP_ = _C_WG + H

_PROG_CACHE = {}


def _build(use_collective: bool):
    import concourse.bacc as bacc
    import concourse.tile as tile
    from concourse import mybir
    from concourse.tile_rust import add_dep_helper

    AF = mybir.ActivationFunctionType
    ALU = mybir.AluOpType
    f32 = mybir.dt.float32
    bf16 = mybir.dt.bfloat16

    nc = bacc.Bacc(
        "TRN2",
        target_bir_lowering=False,
        debug=False,
        num_devices=NCORES,
    )

    qcols = SHARD if use_collective else SEQ
    qs = nc.dram_tensor("qs", [H, qcols], bf16, kind="ExternalInput").ap()
    ks = nc.dram_tensor("ks", [H, SHARD], bf16, kind="ExternalInput").ap()
    Pc = nc.dram_tensor("P", [128, NP_], f32, kind="ExternalInput").ap()
    out = nc.dram_tensor("out", [1, SHARD], f32, kind="ExternalOutput").ap()

    with tile.TileContext(nc) as tc:
        with (
            tc.tile_pool(name="consts", bufs=1) as cp,
            tc.tile_pool(name="work", bufs=1) as wp,
            tc.tile_pool(name="qstream", bufs=NCH if use_collective else 3) as qp,
            tc.tile_pool(name="kstream", bufs=NCH) as kp,
            tc.tile_pool(name="ps_small", bufs=2, space="PSUM") as pps,
            tc.tile_pool(name="ps_keep", bufs=1, space="PSUM") as ppk,
            tc.tile_pool(name="ps_big", bufs=1, space="PSUM") as ppb,
            tc.tile_pool(name="dram", bufs=1, space="DRAM") as dp,
        ):
            # ---- query chunk loads get the wire first: they feed the
            # qmean partial sums and the collective, the head of the
            # critical path.  qtile[c][p, s] = q[s, c*128+p].
            qv = qs.rearrange("(c p) s -> c p s", p=128)
            qtiles = []
            for c in range(NCH):
                qt = qp.tile([128, qcols], bf16, tag="qt")
                nc.sync.dma_start(qt, qv[c])
                qtiles.append(qt)

            # warm the ACT tables used on the post-collective tail
            warm1 = wp.tile([1, 1], f32, tag="w1")
            nc.scalar.activation(warm1, qtiles[0][0:1, 0:1], AF.Tanh)
            warm2 = wp.tile([1, 1], f32, tag="w2")
            nc.scalar.activation(warm2, qtiles[0][0:1, 0:1], AF.Sigmoid)

            # ---- per-chunk query column sums, pipelined with the loads:
            # qmTp[p, c] = sum_s q[s, c*128+p]  (f32 accumulate)
            qmTp = wp.tile([128, NCH], f32, tag="qmTp")
            for c in range(NCH):
                nc.vector.tensor_reduce(
                    qmTp[:, c : c + 1],
                    qtiles[c],
                    axis=mybir.AxisListType.X,
                    op=ALU.add,
                )

            cc_inst = None
            if use_collective:
                cc_in = dp.tile([128, NCH], f32)
                cc_out = dp.tile([NCORES, 128 * NCH], f32)
                cc_inst = nc.scalar.dma_start(cc_in, qmTp)
                nc.gpsimd.collective_compute(
                    "AllGather",
                    ALU.bypass,
                    replica_groups=[list(range(NCORES))],
                    ins=[cc_in.opt()],
                    outs=[cc_out.opt()],
                )

            # ---- packed constants: one DMA, after cc_in on this ring ----
            P = cp.tile([128, NP_], f32)
            nc.scalar.dma_start(P, Pc)
            Wt1T = P[0:32, _C_WT1 : _C_WT1 + 1]
            bt1T = P[0:32, _C_BT1 : _C_BT1 + 1]
            tb = P[0:32, _C_TS : _C_TS + K]
            Wt2 = P[0:32, _C_WT2 : _C_WT2 + F]
            bt2T = P[:, _C_BT2 : _C_BT2 + 1]
            dgT = P[:, _C_DGT : _C_DGT + K]
            ba1T = P[:, _C_BA1 : _C_BA1 + 1]
            ba2c = P[0:K, _C_BA2 : _C_BA2 + 1]
            bgT = P[:, _C_BGT : _C_BGT + NCH]
            Wa2c = P[:, _C_WA2 : _C_WA2 + 1]
            Wa1m = P[:, _C_WA1M : _C_WA1M + F]

            # ---- key stream; ordered after cc_in so the tiny collective
            # input is not stuck behind 2MB of key reads
            kv = ks.rearrange("(c p) s -> c p s", p=128)
            ktiles = []
            for c in range(NCH):
                kt = kp.tile([128, SHARD], bf16, tag="kt")
                ki = nc.sync.dma_start(kt, kv[c])
                if cc_inst is not None and c == 0:
                    add_dep_helper(ki.ins, cc_inst.ins, reason="keys after cc_in")
                ktiles.append(kt)

            # ---- temporal MLP -> memT [F, K] (f32) ----
            h1T = wp.tile([32, K], f32, tag="h1T")
            nc.vector.tensor_scalar_mul(h1T, tb, Wt1T)
            nc.vector.tensor_scalar_add(h1T, h1T, bt1T)
            nc.vector.tensor_relu(h1T, h1T)
            tT_ps = pps.tile([F, K], f32, tag="tmp")
            nc.tensor.matmul(tT_ps, lhsT=Wt2, rhs=h1T, start=True, stop=True)
            memT = wp.tile([F, K], f32, tag="memT")
            nc.scalar.activation(memT, tT_ps, AF.Identity, bias=bt2T, scale=1.0)
            nc.vector.tensor_add(memT, memT, dgT)

            # ---- mem half of the scorer (pre-collective) ----
            haT_ps = ppk.tile([F, K], f32, tag="haT")
            nc.tensor.matmul(haT_ps, lhsT=Wa1m, rhs=memT, start=True, stop=True)

            # ---- gate columns gT[c][p, k] = g_k[c*128+p] in bf16
            # (1/K and bg folded on the host)
            gt_bf = []
            for c in range(NCH):
                g_ps = pps.tile([F, K], f32, tag="gt")
                nc.tensor.matmul(
                    g_ps,
                    lhsT=P[:, _C_WG + c * 128 : _C_WG + (c + 1) * 128],
                    rhs=memT,
                    start=True,
                    stop=True,
                )
                gb = wp.tile([F, K], bf16, tag=f"gb{c}")
                nc.vector.tensor_scalar_add(gb, g_ps, bgT[:, c : c + 1])
                gt_bf.append(gb)

            # ---- matvec: row_ps[k, s] = g_k . key[s], chunked over h ----
            row_ps = ppb.tile([K, SHARD], f32, tag="big")
            for half in range(2):
                sl = slice(half * 512, (half + 1) * 512)
                for c in range(NCH):
                    nc.tensor.matmul(
                        row_ps[:, sl],
                        lhsT=gt_bf[c],
                        rhs=ktiles[c][:, sl],
                        start=(c == 0),
                        stop=(c == NCH - 1),
                    )
            row_sb = wp.tile([K, SHARD], bf16, tag="row")
            nc.scalar.copy(row_sb, row_ps)

            # ---- post-collective: qmT = sum of per-core partials ----
            if use_collective:
                qmTd8 = wp.tile([128, NCORES, NCH], f32, tag="qmTd8")
                nc.sync.dma_start(
                    qmTd8, cc_out[:, :].rearrange("d (p c) -> p d c", c=NCH)
                )
                qmT = wp.tile([128, NCH], f32, tag="qmT")
                nc.vector.tensor_reduce(
                    qmT,
                    qmTd8.rearrange("p d c -> p c d"),
                    axis=mybir.AxisListType.X,
                    op=ALU.add,
                )
            else:
                qmT = qmTp

            # ---- hq[f] = (qmean @ Wa1q)[f]  (1/S folded into Wa1q) ----
            hq_ps = ppk.tile([F, 1], f32, tag="hq")
            for c in range(NCH):
                nc.tensor.matmul(
                    hq_ps,
                    lhsT=P[:, _C_WA1Q + c * 128 : _C_WA1Q + (c + 1) * 128],
                    rhs=qmT[:, c : c + 1],
                    start=(c == 0),
                    stop=(c == NCH - 1),
                )
            hq_sb = wp.tile([F, 1], f32, tag="hq_sb")
            nc.vector.tensor_scalar_add(hq_sb, hq_ps, ba1T)

            # ---- scorer tail: tanh, score, sigmoid ----
            aT = wp.tile([F, K], f32, tag="aT")
            nc.scalar.activation(aT, haT_ps, AF.Tanh, bias=hq_sb, scale=1.0)
            score_ps = pps.tile([K, 1], f32, tag="tmp")
            nc.tensor.matmul(score_ps, lhsT=aT, rhs=Wa2c, start=True, stop=True)
            wcol = wp.tile([K, 1], bf16, tag="wcol")
            nc.scalar.activation(wcol, score_ps, AF.Sigmoid, bias=ba2c, scale=1.0)

            # ---- combine anchors: o_row = wcol^T @ row  (1/K in row) ----
            orow_ps = ppb.tile([K, SHARD], f32, tag="big")
            for half in range(2):
                sl = slice(half * 512, (half + 1) * 512)
                nc.tensor.matmul(
                    orow_ps[0:1, sl],
                    lhsT=wcol,
                    rhs=row_sb[:, sl],
                    start=True,
                    stop=True,
                )
            orow_sb = wp.tile([1, SHARD], f32, tag="orow")
            nc.scalar.copy(orow_sb, orow_ps[0:1, :])
            nc.sync.dma_start(out, orow_sb)

    nc.compile()
    return nc


def _get_prog(use_collective: bool):
    key = bool(use_collective)
    if key not in _PROG_CACHE:
        _PROG_CACHE[key] = _build(key)
    return _PROG_CACHE[key]


def _pack_consts(inputs) -> np.ndarray:
    f = lambda name: np.asarray(inputs[name], np.float32)
    P = np.zeros((128, NP_), np.float32)
    P[0:32, _C_WT1] = f("Wt1")[0]
    P[0:32, _C_BT1] = f("bt1")
    P[0:32, _C_TS : _C_TS + K] = np.broadcast_to(f("timestamps")[None, :], (32, K))
    P[0:32, _C_WT2 : _C_WT2 + F] = f("Wt2")
    P[:, _C_BT2] = f("bt2")
    P[:, _C_DGT : _C_DGT + K] = f("dg_features").T
    P[:, _C_BA1] = f("ba1")
    P[0:K, _C_BA2] = f("ba2")[0]
    P[:, _C_BGT : _C_BGT + NCH] = (f("bg") * (1.0 / K)).reshape(NCH, 128).T
    P[:, _C_WA2] = f("Wa2")[:, 0]
    P[:, _C_WA1M : _C_WA1M + F] = f("Wa1")[0:F, :]
    P[:, _C_WA1Q : _C_WA1Q + H] = (
        (f("Wa1")[F:, :] * (1.0 / SEQ))
        .reshape(NCH, 128, F)
        .transpose(1, 0, 2)
        .reshape(128, NCH * F)
    )
    P[:, _C_WG : _C_WG + H] = f("Wg") * (1.0 / K)
    return np.ascontiguousarray(P)


def _make_in_maps(inputs, use_collective: bool):
    import ml_dtypes

    bf16 = ml_dtypes.bfloat16
    q = np.asarray(inputs["query"], np.float32)[0]  # [S,H]
    k = np.asarray(inputs["key"], np.float32)[0]  # [S,H]
    P = _pack_consts(inputs)
    qb = q.astype(bf16)
    kb = k.astype(bf16)
    if not use_collective:
        q_full = np.ascontiguousarray(qb.T)  # [H, S]
    in_maps = []
    for d in range(NCORES):
        sl = slice(d * SHARD, (d + 1) * SHARD)
        m = {
            "P": P,
            "ks": np.ascontiguousarray(kb[sl].T),  # [H, SHARD]
            "qs": (
                np.ascontiguousarray(qb[sl].T) if use_collective else q_full
            ),
        }
        in_maps.append(m)
    return in_maps


def _run(inputs, use_collective: bool, trace: bool = False):
    from concourse.bass_utils import run_bass_kernel_spmd

    nc = _get_prog(use_collective)
    in_maps = _make_in_maps(inputs, use_collective)
    res = run_bass_kernel_spmd(
        nc, in_maps, core_ids=list(range(NCORES)), trace=trace
    )
    row = np.empty((SEQ,), np.float32)
    for d in range(NCORES):
        row[d * SHARD : (d + 1) * SHARD] = res.results[d]["out"][0]
    # every row of the [S, S] output is the same gate row
    full = np.empty((1, 1, SEQ, SEQ), np.float32)
    full[0, 0, :, :] = row[None, :]
    return full, res


def kernel(**inputs) -> np.ndarray:
    use_collective = os.environ.get("CA1_NO_COLLECTIVE", "0") != "1"
    try:
        full, _ = _run(inputs, use_collective)
        return full
    except Exception:
        if not use_collective:
            raise
        # fall back to the zero-communication variant (replicated query)
        _PROG_CACHE.pop(True, None)
        full, _ = _run(inputs, False)
        return full


# revision 6
# speedup vs baseline: 3.9882x; 1.1403x over previous
"""Trainium2 Bass kernel for nn_CA1AttentionGate.

Computes, for full inputs (B=1, S=8192, H=1024, F=128, K=2):
    temporal = relu(t @ Wt1 + bt1) @ Wt2 + bt2          [K,F]
    mem      = dg_features + temporal                    [K,F]
    qmean    = query.mean(axis=1)                        [1,H]
    score_k  = tanh([mem_k ; qmean] @ Wa1 + ba1) @ Wa2 + ba2
    w_k      = sigmoid(score_k)
    g_k      = mem_k @ Wg + bg                           [K,H]
    row[s]   = (1/K) * sum_k w_k * (g_k . key[s])        [S]
    out      = broadcast(row) -> [1,1,S,S]

Sharding: sequence-parallel over the key/seq axis across 8 cores.  Every
row of the [S,S] output is the same vector, so each core computes only
its 1024-entry slice of that broadcast row from its key shard (the
sharding_hint's "slice of the broadcast row") and the host unshard step
expands the gathered row to the full output.  The only cross-core
quantity is qmean: each core reduces its query shard to per-chunk column
sums and a 4KB AllGather completes the mean (fallback variant replicates
the full query read).

Inputs are staged transposed ([H, shard]) and in bf16 so the query
column-sums are free-axis DVE reduces and the g.key matvec is a chain of
bf16 PE matmuls accumulating in PSUM; 1/K is folded into Wg/bg and 1/S
into the qmean rows of Wa1 on the host.
"""

import os

import numpy as np

SEQ = 8192
H = 1024
F = 128
K = 2
NCORES = 8
SHARD = SEQ // NCORES  # 1024
NCH = H // 128  # 8 h-chunks of 128

# packed f32 constant tensor [128, NP_] column layout
_C_WT1 = 0
_C_BT1 = 1
_C_TS = 2
_C_WT2 = 4
_C_BT2 = 132
_C_DGT = 133
_C_BA1 = 135
_C_BA2 = 136
_C_BGT = 137
_C_WA2 = 145
_C_WA1M = 146
_C_WA1Q = 274
_C_WG = 274 + H
NP_ = _C_WG + H

_PROG_CACHE = {}


def _build(use_collective: bool):
    import concourse.bacc as bacc
    import concourse.tile as tile
    from concourse import mybir
    from concourse.tile_rust import add_dep_helper

    AF = mybir.ActivationFunctionType
    ALU = mybir.AluOpType
    f32 = mybir.dt.float32
    bf16 = mybir.dt.bfloat16

    nc = bacc.Bacc(
        "TRN2",
        target_bir_lowering=False,
        debug=False,
        num_devices=NCORES,
    )

    qcols = SHARD if use_collective else SEQ
    qs = nc.dram_tensor("qs", [H, qcols], bf16, kind="ExternalInput").ap()
    ks = nc.dram_tensor("ks", [H, SHARD], bf16, kind="ExternalInput").ap()
    Pc = nc.dram_tensor("P", [128, NP_], f32, kind="ExternalInput").ap()
    out = nc.dram_tensor("out", [1, SHARD], f32, kind="ExternalOutput").ap()

    with tile.TileContext(nc) as tc:
        with (
            tc.tile_pool(name="consts", bufs=1) as cp,
            tc.tile_pool(name="work", bufs=1) as wp,
            tc.tile_pool(name="qstream", bufs=NCH if use_collective else 3) as qp,
            tc.tile_pool(name="kstream", bufs=NCH) as kp,
            tc.tile_pool(name="ps_small", bufs=2, space="PSUM") as pps,
            tc.tile_pool(name="ps_keep", bufs=1, space="PSUM") as ppk,
            tc.tile_pool(name="ps_big", bufs=1, space="PSUM") as ppb,
            tc.tile_pool(name="dram", bufs=1, space="DRAM") as dp,
        ):
            # ---- query chunk loads get the wire first: they feed the
            # qmean partial sums and the collective, the head of the
            # critical path.  qtile[c][p, s] = q[s, c*128+p].
            qv = qs.rearrange("(c p) s -> c p s", p=128)
            qtiles, q_insts = [], []
            for c in range(NCH):
                qt = qp.tile([128, qcols], bf16, tag="qt")
                q_insts.append(nc.sync.dma_start(qt, qv[c]))
                qtiles.append(qt)

            # ---- per-chunk query column sums, pipelined with the loads
            # and split across DVE (reduce) and ACT (copy+accum) so
            # neither engine chain lags the arriving tiles:
            # qmTp[p, c] = sum_s q[s, c*128+p]  (f32 accumulate)
            qmTp = wp.tile([128, NCH], f32, tag="qmTp")
            junk = wp.tile([128, qcols], bf16, tag="junk")
            for c in range(NCH):
                if c % 2 == 0:
                    nc.vector.tensor_reduce(
                        qmTp[:, c : c + 1],
                        qtiles[c],
                        axis=mybir.AxisListType.X,
                        op=ALU.add,
                    )
                else:
                    nc.scalar.activation(
                        junk,
                        qtiles[c],
                        AF.Copy,
                        accum_out=qmTp[:, c : c + 1],
                    )

            cc_inst = None
            if use_collective:
                cc_in = dp.tile([128, NCH], f32)
                cc_out = dp.tile([NCORES, 128 * NCH], f32)
                cc_inst = nc.scalar.dma_start(cc_in, qmTp)
                nc.gpsimd.collective_compute(
                    "AllGather",
                    ALU.bypass,
                    replica_groups=[list(range(NCORES))],
                    ins=[cc_in.opt()],
                    outs=[cc_out.opt()],
                )

            # warm the ACT tables used on the post-collective tail while
            # the collective is in flight (table loads have no data deps,
            # so they run as soon as the accum chain frees the engine)
            warm1 = wp.tile([1, 1], f32, tag="w1")
            nc.scalar.activation(warm1, qtiles[0][0:1, 0:1], AF.Tanh)
            warm2 = wp.tile([1, 1], f32, tag="w2")
            nc.scalar.activation(warm2, qtiles[0][0:1, 0:1], AF.Sigmoid)

            # ---- packed constants: one DMA, kept off the wire until the
            # tiny collective input is out (it feeds only the k-side work
            # that hides under the collective)
            P = cp.tile([128, NP_], f32)
            p_inst = nc.scalar.dma_start(P, Pc)
            add_dep_helper(
                p_inst.ins,
                (cc_inst if cc_inst is not None else q_insts[-1]).ins,
                reason="consts after cc_in",
            )
            Wt1T = P[0:32, _C_WT1 : _C_WT1 + 1]
            bt1T = P[0:32, _C_BT1 : _C_BT1 + 1]
            tb = P[0:32, _C_TS : _C_TS + K]
            Wt2 = P[0:32, _C_WT2 : _C_WT2 + F]
            bt2T = P[:, _C_BT2 : _C_BT2 + 1]
            dgT = P[:, _C_DGT : _C_DGT + K]
            ba1T = P[:, _C_BA1 : _C_BA1 + 1]
            ba2c = P[0:K, _C_BA2 : _C_BA2 + 1]
            bgT = P[:, _C_BGT : _C_BGT + NCH]
            Wa2c = P[:, _C_WA2 : _C_WA2 + 1]
            Wa1m = P[:, _C_WA1M : _C_WA1M + F]

            # ---- key stream; ordered after cc_in so the tiny collective
            # input is not stuck behind 2MB of key reads
            kv = ks.rearrange("(c p) s -> c p s", p=128)
            ktiles = []
            for c in range(NCH):
                kt = kp.tile([128, SHARD], bf16, tag="kt")
                ki = nc.sync.dma_start(kt, kv[c])
                add_dep_helper(
                    ki.ins,
                    (cc_inst if cc_inst is not None else q_insts[-1]).ins,
                    reason="keys after cc_in",
                )
                ktiles.append(kt)

            # ---- temporal MLP -> memT [F, K] (f32) ----
            h1T = wp.tile([32, K], f32, tag="h1T")
            nc.vector.tensor_scalar_mul(h1T, tb, Wt1T)
            nc.vector.tensor_scalar_add(h1T, h1T, bt1T)
            nc.vector.tensor_relu(h1T, h1T)
            tT_ps = pps.tile([F, K], f32, tag="tmp")
            nc.tensor.matmul(tT_ps, lhsT=Wt2, rhs=h1T, start=True, stop=True)
            memT = wp.tile([F, K], f32, tag="memT")
            nc.scalar.activation(memT, tT_ps, AF.Identity, bias=bt2T, scale=1.0)
            nc.vector.tensor_add(memT, memT, dgT)

            # ---- mem half of the scorer (pre-collective) ----
            haT_ps = ppk.tile([F, K], f32, tag="haT")
            nc.tensor.matmul(haT_ps, lhsT=Wa1m, rhs=memT, start=True, stop=True)

            # ---- gate columns gT[c][p, k] = g_k[c*128+p] in bf16
            # (1/K and bg folded on the host)
            gt_bf = []
            for c in range(NCH):
                g_ps = pps.tile([F, K], f32, tag="gt")
                nc.tensor.matmul(
                    g_ps,
                    lhsT=P[:, _C_WG + c * 128 : _C_WG + (c + 1) * 128],
                    rhs=memT,
                    start=True,
                    stop=True,
                )
                gb = wp.tile([F, K], bf16, tag=f"gb{c}")
                nc.vector.tensor_scalar_add(gb, g_ps, bgT[:, c : c + 1])
                gt_bf.append(gb)

            # ---- matvec: row_ps[k, s] = g_k . key[s], chunked over h ----
            row_ps = ppb.tile([K, SHARD], f32, tag="big")
            for half in range(2):
                sl = slice(half * 512, (half + 1) * 512)
                for c in range(NCH):
                    nc.tensor.matmul(
                        row_ps[:, sl],
                        lhsT=gt_bf[c],
                        rhs=ktiles[c][:, sl],
                        start=(c == 0),
                        stop=(c == NCH - 1),
                    )
            row_sb = wp.tile([K, SHARD], bf16, tag="row")
            nc.scalar.copy(row_sb, row_ps)

            # ---- post-collective: qmT = sum of per-core partials ----
            if use_collective:
                qmTd8 = wp.tile([128, NCORES, NCH], f32, tag="qmTd8")
                nc.sync.dma_start(
                    qmTd8, cc_out[:, :].rearrange("d (p c) -> p d c", c=NCH)
                )
                qmT = wp.tile([128, NCH], f32, tag="qmT")
                nc.vector.tensor_reduce(
                    qmT,
                    qmTd8.rearrange("p d c -> p c d"),
                    axis=mybir.AxisListType.X,
                    op=ALU.add,
                )
            else:
                qmT = qmTp

            # ---- hq[f] = (qmean @ Wa1q)[f]  (1/S folded into Wa1q) ----
            hq_ps = ppk.tile([F, 1], f32, tag="hq")
            for c in range(NCH):
                nc.tensor.matmul(
                    hq_ps,
                    lhsT=P[:, _C_WA1Q + c * 128 : _C_WA1Q + (c + 1) * 128],
                    rhs=qmT[:, c : c + 1],
                    start=(c == 0),
                    stop=(c == NCH - 1),
                )
            hq_sb = wp.tile([F, 1], f32, tag="hq_sb")
            nc.vector.tensor_scalar_add(hq_sb, hq_ps, ba1T)

            # ---- scorer tail: tanh, score, sigmoid ----
            aT = wp.tile([F, K], f32, tag="aT")
            nc.scalar.activation(aT, haT_ps, AF.Tanh, bias=hq_sb, scale=1.0)
            score_ps = pps.tile([K, 1], f32, tag="tmp")
            nc.tensor.matmul(score_ps, lhsT=aT, rhs=Wa2c, start=True, stop=True)
            wcol = wp.tile([K, 1], bf16, tag="wcol")
            nc.scalar.activation(wcol, score_ps, AF.Sigmoid, bias=ba2c, scale=1.0)

            # ---- combine anchors: o_row = wcol^T @ row  (1/K in row);
            # the PSUM->SBUF copy of each half overlaps the other half's
            # matmul (ACT vs PE)
            orow_ps = ppb.tile([K, SHARD], f32, tag="big")
            orow_sb = wp.tile([1, SHARD], f32, tag="orow")
            for half in range(2):
                sl = slice(half * 512, (half + 1) * 512)
                nc.tensor.matmul(
                    orow_ps[0:1, sl],
                    lhsT=wcol,
                    rhs=row_sb[:, sl],
                    start=True,
                    stop=True,
                )
                nc.scalar.copy(orow_sb[:, sl], orow_ps[0:1, sl])
            nc.sync.dma_start(out, orow_sb)

    nc.compile()
    return nc


def _get_prog(use_collective: bool):
    key = bool(use_collective)
    if key not in _PROG_CACHE:
        _PROG_CACHE[key] = _build(key)
    return _PROG_CACHE[key]


def _pack_consts(inputs) -> np.ndarray:
    f = lambda name: np.asarray(inputs[name], np.float32)
    P = np.zeros((128, NP_), np.float32)
    P[0:32, _C_WT1] = f("Wt1")[0]
    P[0:32, _C_BT1] = f("bt1")
    P[0:32, _C_TS : _C_TS + K] = np.broadcast_to(f("timestamps")[None, :], (32, K))
    P[0:32, _C_WT2 : _C_WT2 + F] = f("Wt2")
    P[:, _C_BT2] = f("bt2")
    P[:, _C_DGT : _C_DGT + K] = f("dg_features").T
    P[:, _C_BA1] = f("ba1")
    P[0:K, _C_BA2] = f("ba2")[0]
    P[:, _C_BGT : _C_BGT + NCH] = (f("bg") * (1.0 / K)).reshape(NCH, 128).T
    P[:, _C_WA2] = f("Wa2")[:, 0]
    P[:, _C_WA1M : _C_WA1M + F] = f("Wa1")[0:F, :]
    P[:, _C_WA1Q : _C_WA1Q + H] = (
        (f("Wa1")[F:, :] * (1.0 / SEQ))
        .reshape(NCH, 128, F)
        .transpose(1, 0, 2)
        .reshape(128, NCH * F)
    )
    P[:, _C_WG : _C_WG + H] = f("Wg") * (1.0 / K)
    return np.ascontiguousarray(P)


def _make_in_maps(inputs, use_collective: bool):
    import ml_dtypes

    bf16 = ml_dtypes.bfloat16
    q = np.asarray(inputs["query"], np.float32)[0]  # [S,H]
    k = np.asarray(inputs["key"], np.float32)[0]  # [S,H]
    P = _pack_consts(inputs)
    qb = q.astype(bf16)
    kb = k.astype(bf16)
    if not use_collective:
        q_full = np.ascontiguousarray(qb.T)  # [H, S]
    in_maps = []
    for d in range(NCORES):
        sl = slice(d * SHARD, (d + 1) * SHARD)
        m = {
            "P": P,
            "ks": np.ascontiguousarray(kb[sl].T),  # [H, SHARD]
            "qs": (
                np.ascontiguousarray(qb[sl].T) if use_collective else q_full
            ),
        }
        in_maps.append(m)
    return in_maps


def _run(inputs, use_collective: bool, trace: bool = False):
    from concourse.bass_utils import run_bass_kernel_spmd

    nc = _get_prog(use_collective)
    in_maps = _make_in_maps(inputs, use_collective)
    res = run_bass_kernel_spmd(
        nc, in_maps, core_ids=list(range(NCORES)), trace=trace
    )
    row = np.empty((SEQ,), np.float32)
    for d in range(NCORES):
        row[d * SHARD : (d + 1) * SHARD] = res.results[d]["out"][0]
    # every row of the [S, S] output is the same gate row
    full = np.empty((1, 1, SEQ, SEQ), np.float32)
    full[0, 0, :, :] = row[None, :]
    return full, res


def kernel(**inputs) -> np.ndarray:
    use_collective = os.environ.get("CA1_NO_COLLECTIVE", "0") != "1"
    try:
        full, _ = _run(inputs, use_collective)
        return full
    except Exception:
        if not use_collective:
            raise
        # fall back to the zero-communication variant (replicated query)
        _PROG_CACHE.pop(True, None)
        full, _ = _run(inputs, False)
        return full
